# revision 30
# baseline (speedup 1.0000x reference)
"""FedGATConv forward kernel for Trainium2 (Bass/Tile), 8-core data-parallel.

Computation per node n (N=4096, F=128, S=16, P=9):
  D[n,s]   = att1 . M1[n,:,s] + att2 . M2[n,:,s]
  w[n,p,s] = polycoeffs[p] * D[n,s]^p
  G[n,f]   = sum_{p,s} w[n,p,s] * K1[n,p,s,f]
  E        = G @ weight ; Fden[n] = sum_{p,s} w[n,p,s]*K2[n,p,s]
  out      = E / Fden[:,None]

Sharding: pure data-parallel over nodes, 512 nodes/core, no collectives.

HW model driving the design (measured on this part):
  - 512B DMA descriptors (any ps-on-partition K1 layout) cap aggregate DMA
    at ~300 GB/s; natural-layout loads (36KB descriptors) reach ~362 GB/s.
    ALL loads are therefore natural-layout: DMA floor ~129 us/core.
  - With K1 natural ([node, (c f)], c = p*16+s), the weight w[n, c] is a
    per-partition scalar for fixed c; the c-reduction is done on the
    otherwise-idle PE by accumulating  gt += slice^T @ I  bf16 matmuls.
  - Per-op fixed costs dominate small DVE/ACT ops, so slices are scaled in
    16-slice chunks with ONE tensor_tensor per chunk (weights broadcast
    along f with a step-0 AP); ~138 ns/slice on DVE, ~2x that on gpsimd.
    Chunks are spread DVE:gpsimd:ACT to keep every engine under the DMA
    shadow (ACT takes one chunk as 16 single-slice muls).
  - fp32 PE matmul is ~4x slower + no FWL; all PE operands here are bf16
    except the final small E matmul.
"""

import os
import numpy as np

DEBUG_TAPS = bool(os.environ.get("KERNEL_DEBUG_TAPS"))

N_FULL = 4096
F = 128          # IN_FEAT == OUT_FEAT
S = 16
P = 9
PS = P * S       # 144 = number of (p, s) columns c
NCORES = 8
NS = N_FULL // NCORES   # 512 nodes per core
BLK = 128               # nodes per block
NBLK = NS // BLK        # 4
CCHUNK = 16             # c-slices per scale/fold chunk (= c-columns per K1 piece)
NCHUNK = PS // CCHUNK   # 9 chunks per block
# chunk -> engine: V=vector, G=gpsimd, A=scalar(ACT)
CHUNK_ENG = "VVAVGVAGV"

_BUILT = None


def _build():
    """Build and return the compiled Bass module (cached per process)."""
    global _BUILT
    if _BUILT is not None:
        return _BUILT

    import concourse.bacc as bacc
    import concourse.tile as tile
    import concourse.mybir as mybir
    from concourse import masks

    f32 = mybir.dt.float32
    bf16 = mybir.dt.bfloat16

    nc = bacc.Bacc("TRN2", target_bir_lowering=False, debug=False)

    M1d = nc.dram_tensor("M1", [NS, F, S], f32, kind="ExternalInput").ap()
    M2d = nc.dram_tensor("M2", [NS, F, S], f32, kind="ExternalInput").ap()
    K1d = nc.dram_tensor("K1", [NS, P, S, F], f32, kind="ExternalInput").ap()
    K2d = nc.dram_tensor("K2", [NS, P, S], f32, kind="ExternalInput").ap()
    att1d = nc.dram_tensor("att1", [F], f32, kind="ExternalInput").ap()
    att2d = nc.dram_tensor("att2", [F], f32, kind="ExternalInput").ap()
    wtd = nc.dram_tensor("weight", [F, F], f32, kind="ExternalInput").ap()
    polyd = nc.dram_tensor("polycoeffs", [P], f32, kind="ExternalInput").ap()
    outd = nc.dram_tensor("out", [NS, F], f32, kind="ExternalOutput").ap()
    if DEBUG_TAPS:
        dbg_d = nc.dram_tensor("dbg_d", [NS, S], f32, kind="ExternalOutput").ap()
        dbg_fden = nc.dram_tensor("dbg_fden", [NS, 1], f32, kind="ExternalOutput").ap()
        dbg_gt = nc.dram_tensor("dbg_gt", [NBLK, F, BLK], f32, kind="ExternalOutput").ap()

    K1ps = K1d.rearrange("n p s f -> n (p s) f")   # [NS, 144, 128]
    K2ps = K2d.rearrange("n p s -> n (p s)")       # [NS, 144]

    with tile.TileContext(nc) as tc:
        with (
            tc.tile_pool(name="const", bufs=1) as cpool,
            tc.tile_pool(name="m12", bufs=2) as mpool,
            tc.tile_pool(name="kp", bufs=18) as kppool,
            tc.tile_pool(name="scb", bufs=4) as scpool,
            tc.tile_pool(name="small", bufs=2) as spool,
            tc.tile_pool(name="pw", bufs=2) as pwpool,
            tc.tile_pool(name="outp", bufs=4) as outpool,
            tc.tile_pool(name="ps_gt", bufs=2, space="PSUM") as psgt,
            tc.tile_pool(name="ps_e", bufs=1, space="PSUM") as pse,
        ):
            # ---------------- constants ----------------
            w_sb = cpool.tile([F, F], f32)            # weight [f, o]
            nc.sync.dma_start(w_sb[:], wtd[:])

            ident = cpool.tile([128, 128], f32)
            masks.make_identity(nc, ident[:])
            ident_bf = cpool.tile([128, 128], bf16)
            nc.vector.tensor_copy(ident_bf[:], ident[:])
            c0_identbf = cpool.tile([128, 128], bf16)

            ones_row = cpool.tile([1, 128], f32)
            nc.vector.memset(ones_row[:], 1.0)
            ones16 = cpool.tile([128, S], f32)
            nc.vector.memset(ones16[:], 1.0)

            poly_row = cpool.tile([1, P], f32)
            nc.sync.dma_start(poly_row[:], polyd.unsqueeze(0))
            att1_row = cpool.tile([1, F], f32)
            att2_row = cpool.tile([1, F], f32)
            nc.sync.dma_start(att1_row[:], att1d.unsqueeze(0))
            nc.sync.dma_start(att2_row[:], att2d.unsqueeze(0))

            poly_ps = pse.tile([128, P], f32, tag="polyps")
            nc.tensor.matmul(poly_ps[:], ones_row[:], poly_row[:],
                             start=True, stop=True)
            poly_rep = cpool.tile([128, P], f32)
            nc.vector.tensor_copy(poly_rep[:], poly_ps[:])
            nc.vector.tensor_scalar(c0_identbf[:], ident[:],
                                    poly_rep[:, 0:1], None,
                                    op0=mybir.AluOpType.mult)

            att1_bc = cpool.tile([128, F], f32)
            att2_bc = cpool.tile([128, F], f32)
            for row, bc in ((att1_row, att1_bc), (att2_row, att2_bc)):
                ps_t = pse.tile([128, F], f32, tag="attps")
                nc.tensor.matmul(ps_t[:], ones_row[:], row[:], start=True, stop=True)
                nc.vector.tensor_copy(bc[:], ps_t[:])

            # ---------------- per-block pipeline ----------------
            # out-stores are deferred two blocks so they never sit ahead of
            # younger loads in the sync ring's FIFO
            pending_stores = []
            for blk in range(NBLK):
                nb = blk * BLK

                if len(pending_stores) >= 2:
                    st_nb, st_tile = pending_stores.pop(0)
                    nc.sync.dma_start(outd[st_nb:st_nb + BLK, :], st_tile[:])

                # -- DMAs: ALL natural layout, split across both HWDGE rings.
                # K1 in 9 chunk-aligned pieces (8KB lines) so each chunk's
                # compute depends only on its own 1MB piece.
                kts = []
                kt0 = kppool.tile([BLK, CCHUNK * F], f32, tag="kp")
                nc.sync.dma_start(
                    kt0[:], K1ps[nb:nb + BLK, 0:CCHUNK, :].rearrange("n c f -> n (c f)"))
                kts.append(kt0)

                m1n = mpool.tile([BLK, F * S], f32, tag="m1")
                m2n = mpool.tile([BLK, F * S], f32, tag="m2")
                nc.sync.dma_start(m1n[:], M1d[nb:nb + BLK].rearrange("n f s -> n (f s)"))
                nc.scalar.dma_start(m2n[:], M2d[nb:nb + BLK].rearrange("n f s -> n (f s)"))

                for t in range(1, NCHUNK):
                    kt = kppool.tile([BLK, CCHUNK * F], f32, tag="kp")
                    issuer = nc.scalar if t % 2 else nc.sync
                    issuer.dma_start(
                        kt[:], K1ps[nb:nb + BLK, t * CCHUNK:(t + 1) * CCHUNK, :]
                        .rearrange("n c f -> n (c f)"))
                    kts.append(kt)

                k2row = spool.tile([BLK, PS], f32, tag="k2")
                nc.sync.dma_start(k2row[:], K2ps[nb:nb + BLK])

                # chunk scale+fold emitter; chunk 0 (p=0) is independent of
                # D and is emitted FIRST to fill the D-computation latency.
                gt_ps = psgt.tile([128, BLK], f32)

                def emit_chunk(ch, wsrc):
                    c0 = ch * CCHUNK
                    kn = kts[ch]
                    sc = scpool.tile([BLK, CCHUNK * F], bf16, tag="sc")
                    eng = CHUNK_ENG[ch]
                    if ch == 0:
                        # p=0: constant weight folded via c0*I rhs; pure cast
                        # on the scalar engine (frees DVE, the block pacer)
                        nc.scalar.copy(sc[:], kn[:])
                    elif eng == "A":
                        for i in range(CCHUNK):
                            nc.scalar.mul(sc[:, i * F:(i + 1) * F],
                                          kn[:, i * F:(i + 1) * F],
                                          wsrc[:, i:i + 1])
                    else:
                        e = nc.vector if eng == "V" else nc.gpsimd
                        wb = wsrc[:, 0:CCHUNK].unsqueeze(2).broadcast_to(
                            [BLK, CCHUNK, F])
                        e.tensor_tensor(
                            out=sc[:].rearrange("n (c f) -> n c f", f=F),
                            in0=kn[:].rearrange("n (c f) -> n c f", f=F),
                            in1=wb, op=mybir.AluOpType.mult)
                    rhs = c0_identbf if ch == 0 else ident_bf
                    for i in range(CCHUNK):
                        c = c0 + i
                        nc.tensor.matmul(gt_ps[:], sc[:, i * F:(i + 1) * F],
                                         rhs[:],
                                         start=(c == 0), stop=(c == PS - 1),
                                         skip_group_check=True)

                emit_chunk(0, None)

                # -- D: M1 branch on DVE, M2 branch on gpsimd (in place) --
                # mult by att (broadcast over s), then f-halves pairwise tree
                # in the (f s) layout (f-halves are contiguous column ranges).
                att1_x = att1_bc[:].unsqueeze(2).broadcast_to([BLK, F, S])
                att2_x = att2_bc[:].unsqueeze(2).broadcast_to([BLK, F, S])
                m1v = m1n[:].rearrange("n (f s) -> n f s", s=S)
                m2v = m2n[:].rearrange("n (f s) -> n f s", s=S)
                d_ns = spool.tile([BLK, S], f32, tag="dns")
                nc.vector.tensor_tensor(out=m1v, in0=m1v, in1=att1_x,
                                        op=mybir.AluOpType.mult)
                nc.gpsimd.tensor_tensor(out=m2v, in0=m2v, in1=att2_x,
                                        op=mybir.AluOpType.mult)
                for eng, mt in ((nc.vector, m1n), (nc.gpsimd, m2n)):
                    fh = (F // 2) * S
                    while fh >= S:
                        eng.tensor_tensor(out=mt[:, 0:fh], in0=mt[:, 0:fh],
                                          in1=mt[:, fh:2 * fh],
                                          op=mybir.AluOpType.add)
                        fh //= 2
                nc.vector.tensor_tensor(out=d_ns[:], in0=m1n[:, 0:S],
                                        in1=m2n[:, 0:S],
                                        op=mybir.AluOpType.add)

                # -- powers D^p -> pow_tile, then one broadcast poly-scale --
                pow_t = pwpool.tile([BLK, (P - 1) * S], f32, tag="pow")
                nc.vector.tensor_copy(pow_t[:, 0:S], d_ns[:])
                for p in range(2, P):
                    nc.vector.tensor_tensor(
                        out=pow_t[:, S * (p - 1):S * p],
                        in0=pow_t[:, S * (p - 2):S * (p - 1)], in1=d_ns[:],
                        op=mybir.AluOpType.mult)
                # w_full[n, j] = c_{1+j//16} * D^(1+j//16)  (j = c - 16)
                w_full = spool.tile([BLK, (P - 1) * S], f32, tag="wfull")
                nc.vector.tensor_tensor(
                    out=w_full[:].rearrange("n (p s) -> n p s", s=S),
                    in0=pow_t[:].rearrange("n (p s) -> n p s", s=S),
                    in1=poly_rep[:, 1:P].unsqueeze(2).broadcast_to(
                        [BLK, P - 1, S]),
                    op=mybir.AluOpType.mult)

                # -- scale+fold the D-dependent chunks 1..8 --
                for ch in range(1, NCHUNK):
                    emit_chunk(ch, w_full[:, ch * CCHUNK - S:])

                # -- Fden = sum_c w[n,c]*K2[n,c] via fused stt+accum --
                v_row = spool.tile([BLK, PS], f32, tag="vrow")
                fden_lo = spool.tile([BLK, 1], f32, tag="fdlo")
                fden_hi = spool.tile([BLK, 1], f32, tag="fdhi")
                nc.vector.scalar_tensor_tensor(
                    out=v_row[:, 0:S], in0=k2row[:, 0:S],
                    scalar=poly_rep[:, 0:1], in1=ones16[:],
                    op0=mybir.AluOpType.mult, op1=mybir.AluOpType.mult,
                    accum_out=fden_lo[:])
                nc.vector.scalar_tensor_tensor(
                    out=v_row[:, S:PS], in0=w_full[:],
                    scalar=1.0, in1=k2row[:, S:PS],
                    op0=mybir.AluOpType.mult, op1=mybir.AluOpType.mult,
                    accum_out=fden_hi[:])
                rec = spool.tile([BLK, 1], f32, tag="rec")
                nc.vector.tensor_tensor(out=rec[:], in0=fden_lo[:], in1=fden_hi[:],
                                        op=mybir.AluOpType.add)
                nc.vector.reciprocal(rec[:], rec[:])

                gt_sb = spool.tile([128, BLK], f32, tag="gtsb")
                nc.scalar.copy(gt_sb[:], gt_ps[:])

                if DEBUG_TAPS:
                    nc.sync.dma_start(dbg_d[nb:nb + BLK, :], d_ns[:])
                    nc.sync.dma_start(dbg_fden[nb:nb + BLK, :], rec[:])
                    nc.sync.dma_start(dbg_gt[blk], gt_sb[:])

                # -- E = gt.T @ weight (fp32), scale rows by 1/Fden --
                e_ps = pse.tile([BLK, F], f32)
                nc.tensor.matmul(e_ps[:], gt_sb[:], w_sb[:], start=True, stop=True)
                out_sb = outpool.tile([BLK, F], f32, tag="outsb")
                nc.scalar.mul(out_sb[:], e_ps[:], rec[:])
                pending_stores.append((nb, out_sb))

            for st_nb, st_tile in pending_stores:
                nc.sync.dma_start(outd[st_nb:st_nb + BLK, :], st_tile[:])

    nc.compile()
    _BUILT = nc
    return nc


def _run_sharded(inputs, trace=False, trace_kwargs=None):
    """Shard inputs over 8 cores, run, gather. Returns (out, BassKernelResults)."""
    from concourse.bass_utils import run_bass_kernel_spmd

    M1 = np.ascontiguousarray(np.asarray(inputs["M1"], dtype=np.float32))
    M2 = np.ascontiguousarray(np.asarray(inputs["M2"], dtype=np.float32))
    K1 = np.ascontiguousarray(np.asarray(inputs["K1"], dtype=np.float32))
    K2 = np.ascontiguousarray(np.asarray(inputs["K2"], dtype=np.float32))
    att1 = np.ascontiguousarray(np.asarray(inputs["att1"], dtype=np.float32))
    att2 = np.ascontiguousarray(np.asarray(inputs["att2"], dtype=np.float32))
    weight = np.ascontiguousarray(np.asarray(inputs["weight"], dtype=np.float32))
    poly = np.ascontiguousarray(np.asarray(inputs["polycoeffs"], dtype=np.float32))

    nc = _build()
    in_maps = []
    for c in range(NCORES):
        lo, hi = c * NS, (c + 1) * NS
        in_maps.append({
            "M1": M1[lo:hi], "M2": M2[lo:hi],
            "K1": K1[lo:hi], "K2": K2[lo:hi],
            "att1": att1, "att2": att2, "weight": weight,
            "polycoeffs": poly,
        })
    kwargs = {}
    if trace:
        kwargs["trace"] = True
        if trace_kwargs:
            kwargs.update(trace_kwargs)
    res = run_bass_kernel_spmd(nc, in_maps, core_ids=list(range(NCORES)), **kwargs)
    out = np.concatenate([res.results[c]["out"] for c in range(NCORES)], axis=0)
    return out, res


def kernel(**inputs):
    out, _ = _run_sharded(inputs, trace=False)
    return out
